# revision 41
# baseline (speedup 1.0000x reference)
"""Trainium2 Bass kernel for nn_Decoder_9715216023709.

Computes (lambda_src [512,20000], lambda_dst [512,20000], return_time_pred [512])
for the Hawkes-process decoder, sharded over 8 NeuronCores:
  - the [B,N] lambda branches are sharded along N (2500 nodes per core,
    per-batch params replicated),
  - the [S,B] time-integration branch is sharded along B (64 rows per core),
  - outputs are disjoint, nothing is all-reduced.

Key algebraic restructure: softplus((a[b] + v_e[n]) * ipsi_e) has a separable
exponential, exp(.) = A[b] * V_e[n] with A = exp(a*ipsi) per batch and
V_e = exp(ipsi_e * v_e) per node (both O(B)/O(N), precomputed on the host).
The per-element device work is then a single ACT Ln pass:
  lambda = psi * Ln(1 + A[b] * V_{et(b)}[n])
with the event-type selection of V done by a K=4 bf16 hi/lo matmul
(masks x [Vhi; Vlo]) and A[b] applied as the ACT per-partition scale.
The same trick removes the per-element exp from the [S,B] intensity.

Device layout per core:
  lambda:  partitions = batch (4 tiles of 128), free = node chunks
           (1024|1024|452, PSUM-bank aligned; matmul N<=512 into slices).
           matmul -> ACT Ln(scale*x+1) -> DVE *psi -> DMA out.
  integ:   partitions = (half, b_local), s split into two 2501-halves.
           F-table [128,2501] from host; ACT Ln, DVE cumsum via
           tensor_tensor_scan, cross-half fixup via a tiny DMA, ACT Exp,
           fused multiply-reduce for the trapezoid sum.
"""

import numpy as np

import concourse.bass as bass
import concourse.mybir as mybir
import concourse.tile as tile
from concourse import bacc
from concourse.bass_utils import run_bass_kernel_spmd

FP = mybir.dt.float32
BF = mybir.dt.bfloat16
AF = mybir.ActivationFunctionType
ALU = mybir.AluOpType

NCORES = 8
N, B, D, S = 20000, 512, 32, 5001
NS = N // NCORES          # 2500 nodes per core
BS = B // NCORES          # 64 batch rows per core (integration branch)
H = 2501                  # integration half length (2*H = 5002 >= S)
NT = B // 128             # 4 batch tiles of 128 partitions
TDM = 5000.0
# lambda free-dim chunks: PSUM-bank aligned (bank = 512 f32)
CHUNKS = [(0, 1024), (1024, 1024), (2048, 452)]

_ONE_ACT_SET = "natural_log_exp_and_others"


def _build_nc() -> bass.Bass:
    nc = bacc.Bacc("TRN2", debug=False, target_bir_lowering=False)

    vexp_s = nc.dram_tensor("vexp_s", [4, NS], BF, kind="ExternalInput")
    vexp_d = nc.dram_tensor("vexp_d", [4, NS], BF, kind="ExternalInput")
    masks4 = nc.dram_tensor("masks4", [4, B], BF, kind="ExternalInput")
    lpar = nc.dram_tensor("lpar", [128, 12], FP, kind="ExternalInput")
    ipar = nc.dram_tensor("ipar", [128, 4], FP, kind="ExternalInput")
    ftab = nc.dram_tensor("ftab", [4, 2 * H], BF, kind="ExternalInput")
    imasks = nc.dram_tensor("imasks", [4, BS], BF, kind="ExternalInput")
    lam_s = nc.dram_tensor("lam_s", [B, NS], FP, kind="ExternalOutput")
    lam_d = nc.dram_tensor("lam_d", [B, NS], FP, kind="ExternalOutput")
    rtp = nc.dram_tensor("rtp", [BS, 1], FP, kind="ExternalOutput")

    with tile.TileContext(nc) as tc:
        with (
            tc.tile_pool(name="const", bufs=1) as cpool,
            tc.tile_pool(name="work", bufs=8) as wpool,
            tc.tile_pool(name="big", bufs=1) as bigpool,
            tc.tile_pool(name="xps", bufs=3, space="PSUM") as ppool,
            tc.tile_pool(name="xps_s", bufs=2, space="PSUM") as pspool,
        ):
            # ---- input loads (SWDGE; small) -------------------------------
            vexp_s_sb = cpool.tile([4, NS], BF, tag="vexp_s")
            nc.sync.dma_start(vexp_s_sb[:], vexp_s[:])
            vexp_d_sb = cpool.tile([4, NS], BF, tag="vexp_d")
            nc.sync.dma_start(vexp_d_sb[:], vexp_d[:])
            masks_sb = cpool.tile([4, B], BF, tag="masks4")
            nc.sync.dma_start(masks_sb[:], masks4[:])
            lpar_sb = cpool.tile([128, 12], FP, tag="lpar")
            nc.sync.dma_start(lpar_sb[:], lpar[:])
            ipar_sb = cpool.tile([128, 4], FP, tag="ipar")
            nc.gpsimd.dma_start(ipar_sb[:], ipar[:])
            ftab_sb = cpool.tile([4, 2 * H], BF, tag="ftab")
            nc.sync.dma_start(ftab_sb[:], ftab[:])
            imasks_sb = cpool.tile([4, BS], BF, tag="imasks")
            nc.sync.dma_start(imasks_sb[:], imasks[:])

            # zero-dep ACT op: hoists the activation-table load to t=0
            warm = cpool.tile([1, 1], FP, tag="warm")
            nc.scalar.activation(warm[:], warm[:], AF.Ln, bias=1.0, scale=0.0)

            # ---- lambda branches ------------------------------------------
            # lpar columns: [A_src(t) x4 | A_dst(t) x4 | psi(t) x4]
            def lam_group(t):
                for c0, cw in CHUNKS:
                    for lamout, vexp_sb, acol0 in ((lam_s, vexp_s_sb, 0), (lam_d, vexp_d_sb, 4)):
                        pool = ppool if cw > 512 else pspool
                        xps = pool.tile([128, cw], FP, tag=f"xps{cw}")
                        for off in range(0, cw, 512):
                            w = min(512, cw - off)
                            nc.tensor.matmul(
                                xps[:, off : off + w],
                                masks_sb[:, t * 128 : (t + 1) * 128],
                                vexp_sb[:, c0 + off : c0 + off + w],
                                start=True,
                                stop=True,
                            )
                        act_sb = wpool.tile([128, cw], FP, tag=f"act{cw}")
                        nc.scalar.activation(
                            act_sb[:],
                            xps[:],
                            AF.Ln,
                            bias=1.0,
                            scale=lpar_sb[:, acol0 + t : acol0 + t + 1],
                        )
                        out_sb = wpool.tile([128, cw], FP, tag=f"out{cw}")
                        nc.vector.tensor_scalar_mul(
                            out_sb[:], act_sb[:], lpar_sb[:, 8 + t : 9 + t]
                        )
                        nc.sync.dma_start(
                            lamout[t * 128 : (t + 1) * 128, c0 : c0 + cw], out_sb[:]
                        )

            lam_group(0)
            # ---- time-integration branch ----------------------------------
            # ipar columns: [G | psi | H*h | unused]
            sbf = bigpool.tile([128, H], FP, tag="sbf")
            nc.gpsimd.iota(sbf[:], pattern=[[1, H]], base=0, channel_multiplier=0,
                           allow_small_or_imprecise_dtypes=True)

            # trapezoid weights wl[s] = s, except wl[5000]=2500, wl[>=5001]=0
            wl = bigpool.tile([128, H], FP, tag="wl")
            nc.gpsimd.tensor_scalar_add(wl[:], sbf[:], ipar_sb[:, 2:3])
            nc.vector.memset(wl[64:128, H - 2 : H - 1], 2500.0)
            nc.vector.memset(wl[64:128, H - 1 : H], 0.0)

            sp = bigpool.tile([128, H], FP, tag="sp")
            for c0, cw in ((0, 1024), (1024, 1024), (2048, 453)):
                cwp = cw if cw % 512 == 0 else ((cw // 512) + 1) * 512
                xt = ppool.tile([128, cwp], FP, tag="xps1024")
                for off in range(0, cw, 512):
                    w = min(512, cw - off)
                    for h in (0, 1):
                        nc.tensor.matmul(
                            xt[64 * h : 64 * h + 64, off : off + w],
                            imasks_sb[:],
                            ftab_sb[:, H * h + c0 + off : H * h + c0 + off + w],
                            start=True,
                            stop=True,
                        )
                nc.scalar.activation(sp[:, c0 : c0 + cw], xt[:, :cw], AF.Ln, bias=1.0,
                                     scale=ipar_sb[:, 0:1])
            intens = bigpool.tile([128, H], FP, tag="int")
            nc.gpsimd.tensor_scalar_mul(intens[:], sp[:], ipar_sb[:, 1:2])

            cum = bigpool.tile([128, H], FP, tag="cum")
            nc.vector.tensor_tensor_scan(
                cum[:], intens[:], intens[:], 0.0, op0=ALU.add, op1=ALU.bypass
            )
            # carry half-0 totals into half-1 rows
            fixt = cpool.tile([128, 1], FP, tag="fixt")
            nc.gpsimd.dma_start(fixt[64:128, :], cum[0:64, H - 1 : H])
            nc.gpsimd.tensor_scalar_add(cum[64:128, :], cum[64:128, :], fixt[64:128, :])

            expc = bigpool.tile([128, H], FP, tag="expc")
            nc.scalar.activation(expc[:], cum[:], AF.Exp, bias=0.0, scale=-1.0)
            wint = bigpool.tile([128, H], FP, tag="wint")
            nc.vector.tensor_mul(wint[:], intens[:], wl[:])
            t2 = bigpool.tile([128, H], FP, tag="t2")
            nc.vector.tensor_mul(t2[:], wint[:], expc[:])
            rsum = cpool.tile([128, 1], FP, tag="rsum")
            nc.vector.tensor_reduce(rsum[:], t2[:], axis=mybir.AxisListType.X, op=ALU.add)
            rtop = cpool.tile([128, 1], FP, tag="rtop")
            nc.gpsimd.dma_start(rtop[0:64, :], rsum[64:128, :])
            rout = cpool.tile([64, 1], FP, tag="rout")
            nc.vector.tensor_add(rout[:], rsum[0:64, :], rtop[0:64, :])
            nc.gpsimd.dma_start(rtp[:], rout[:])


            for t in range(1, NT):
                lam_group(t)

    # Bias activation-table selection to the single exp+ln set so the
    # compiler emits one table load instead of alternating per function.
    import concourse.bacc as bacc_mod
    orig_gat = bacc_mod.get_activation_tables
    def _gat_one_set(arch):
        t = orig_gat(arch)
        return {k: (v if k == _ONE_ACT_SET else set()) for k, v in t.items()}
    bacc_mod.get_activation_tables = _gat_one_set
    try:
        nc.compile()
    finally:
        bacc_mod.get_activation_tables = orig_gat
    return nc


_NC_CACHE: list = []


def _get_nc() -> bass.Bass:
    if not _NC_CACHE:
        _NC_CACHE.append(_build_nc())
    return _NC_CACHE[0]


def _host_prep(all_embeddings, assoc, src, pos_dst, neg_dst, last_update,
               cur_time, et, W0, b0, W1, b1, psi, alpha, w_t):
    import ml_dtypes
    bf16 = ml_dtypes.bfloat16
    f8 = np.float64

    emb = np.asarray(all_embeddings, f8)
    assoc = np.asarray(assoc, np.int64)
    src = np.asarray(src, np.int64)
    pos_dst = np.asarray(pos_dst, np.int64)
    last_update = np.asarray(last_update, f8)
    cur_time = np.asarray(cur_time, f8)
    etb = (np.asarray(et) > 0).astype(np.int64)
    W0 = np.asarray(W0, f8)
    W1 = np.asarray(W1, f8)
    b0f, b1f = float(np.asarray(b0).reshape(-1)[0]), float(np.asarray(b1).reshape(-1)[0])
    psi = np.asarray(psi, f8)
    alpha = np.asarray(alpha, f8)
    w_t = np.asarray(w_t, f8)

    W0u, W0v = W0[0, :D], W0[0, D:]
    W1u, W1v = W1[0, :D], W1[0, D:]

    ipsi2 = 1.0 / (psi + 1e-7)           # per event type [2]
    psi_e, alpha_e, wt_e = psi[etb], alpha[etb], w_t[etb]
    inv_psi = ipsi2[etb]
    z_src = emb[assoc[src]]
    z_dst = emb[assoc[pos_dst]]
    td_src = cur_time - last_update[assoc[src]]
    td_dst = cur_time - last_update[assoc[pos_dst]]

    a_src = np.where(etb == 1, z_src @ W1u + b1f, z_src @ W0u + b0f) \
        + alpha_e * np.exp(-wt_e * td_src / TDM)
    a_dst = np.where(etb == 1, z_dst @ W1v + b1f, z_dst @ W0v + b0f) \
        + alpha_e * np.exp(-wt_e * td_dst / TDM)
    gb = np.where(etb == 1, z_src @ W1u + z_dst @ W1v + b1f,
                  z_src @ W0u + z_dst @ W0v + b0f)

    A_src = np.exp(a_src * inv_psi)
    A_dst = np.exp(a_dst * inv_psi)
    G = np.exp(gb * inv_psi)

    lpar = np.zeros((128, 12), np.float32)
    for t in range(NT):
        sl = slice(128 * t, 128 * (t + 1))
        lpar[:, t] = A_src[sl]
        lpar[:, 4 + t] = A_dst[sl]
        lpar[:, 8 + t] = psi_e[sl]

    m0, m1 = (etb == 0).astype(f8), (etb == 1).astype(f8)
    masks_np = np.stack([m0, m1, m0, m1]).astype(bf16)  # [4, B]

    # per-node exp tables V_e = exp(ipsi_e * (emb @ W)), split hi/lo in bf16
    def _vexp(Wa, Wb):  # event-type 0 uses Wa, 1 uses Wb
        V0 = np.exp(ipsi2[0] * (emb @ Wa))
        V1 = np.exp(ipsi2[1] * (emb @ Wb))
        return V0, V1

    V0s, V1s = _vexp(W0v, W1v)   # lambda_src selects node_v
    V0d, V1d = _vexp(W0u, W1u)   # lambda_dst selects node_u

    # integration F table: F[s,e] = exp(c_e * exp(-w_e*s/TDM)), c_e=ipsi_e*alpha_e
    s_all = np.arange(2 * H, dtype=f8)
    Ftab = np.exp((ipsi2 * alpha)[None, :] * np.exp(-w_t[None, :] * s_all[:, None] / TDM))  # [2H, 2]
    fhi = Ftab.astype(bf16)                          # [2H, 2]
    flo = (Ftab - fhi.astype(f8)).astype(bf16)
    ftab_np = np.stack([fhi[:, 0], fhi[:, 1], flo[:, 0], flo[:, 1]]).astype(bf16)  # [4, 2H]

    def _hilo4(V0, V1, sl):
        rows = np.stack([V0[sl], V1[sl]])                  # [2, NS] f64
        hi = rows.astype(bf16)
        lo = (rows - hi.astype(f8)).astype(bf16)
        return np.concatenate([hi, lo], axis=0)            # [4, NS]: V0hi,V1hi,V0lo,V1lo

    in_maps = []
    for i in range(NCORES):
        nsl = slice(NS * i, NS * (i + 1))
        bsl = slice(BS * i, BS * (i + 1))
        ipar = np.zeros((128, 4), np.float32)
        for h in (0, 1):
            rows = slice(64 * h, 64 * (h + 1))
            ipar[rows, 0] = G[bsl]
            ipar[rows, 1] = psi_e[bsl]
            ipar[rows, 2] = H * h
        im0 = (etb[bsl] == 0).astype(f8)
        im1 = (etb[bsl] == 1).astype(f8)
        imasks_np = np.stack([im0, im1, im0, im1]).astype(bf16)  # [4, BS]
        in_maps.append({
            "vexp_s": _hilo4(V0s, V1s, nsl),
            "vexp_d": _hilo4(V0d, V1d, nsl),
            "masks4": masks_np,
            "lpar": lpar,
            "ipar": ipar,
            "ftab": ftab_np,
            "imasks": imasks_np,
        })
    return in_maps


def kernel(**inputs) -> tuple:
    nc = _get_nc()
    in_maps = _host_prep(**inputs)
    res = run_bass_kernel_spmd(nc, in_maps, core_ids=list(range(NCORES)))
    lam_s = np.concatenate([r["lam_s"] for r in res.results], axis=1)
    lam_d = np.concatenate([r["lam_d"] for r in res.results], axis=1)
    rtp = np.concatenate([r["rtp"][:, 0] for r in res.results])
    return (lam_s, lam_d, rtp)


def kernel_timed(iters=8, **inputs):
    """Dev helper: dispatch the compiled NEFF `iters` times with
    device-resident inputs and report (median_delta_s, times). The delta
    between consecutive async executions approximates per-execution device
    time + dispatch overhead."""
    import time
    import jax
    from jax.sharding import Mesh, PartitionSpec, NamedSharding
    from jax.experimental.shard_map import shard_map
    from concourse import bass2jax

    nc = _get_nc()
    in_maps = _host_prep(**inputs)
    bass2jax.install_neuronx_cc_hook()

    partition_name = nc.partition_id_tensor.name if nc.partition_id_tensor else None
    in_names, out_names, out_avals, zero_outs = [], [], [], []
    import concourse.mybir as mb
    for alloc in nc.m.functions[0].allocations:
        if not isinstance(alloc, mb.MemoryLocationSet):
            continue
        name = alloc.memorylocations[0].name
        if alloc.kind == "ExternalInput":
            if name != partition_name:
                in_names.append(name)
        elif alloc.kind == "ExternalOutput":
            shape = tuple(alloc.tensor_shape)
            dtype = mb.dt.np(alloc.dtype)
            out_names.append(name)
            out_avals.append(jax.core.ShapedArray(shape, dtype))
            zero_outs.append(np.zeros(shape, dtype))
    n_params = len(in_names)
    n_outs = len(out_avals)
    all_in_names = list(in_names) + list(out_names)
    if partition_name is not None:
        all_in_names.append(partition_name)

    def _body(*args):
        operands = list(args)
        if partition_name is not None:
            operands.append(bass2jax.partition_id_tensor())
        outs = bass2jax._bass_exec_p.bind(
            *operands,
            out_avals=tuple(out_avals),
            in_names=tuple(all_in_names),
            out_names=tuple(out_names),
            lowering_input_output_aliases=(),
            sim_require_finite=True,
            sim_require_nnan=True,
            nc=nc,
        )
        return tuple(outs)

    devices = jax.devices()[:NCORES]
    mesh = Mesh(np.asarray(devices), ("core",))
    spec = PartitionSpec("core")
    sharded = jax.jit(
        shard_map(_body, mesh=mesh,
                  in_specs=(spec,) * (n_params + n_outs),
                  out_specs=(spec,) * n_outs, check_rep=False),
        donate_argnums=tuple(range(n_params, n_params + n_outs)),
        keep_unused=True,
    )
    shard = NamedSharding(mesh, spec)
    concat_in = [
        jax.device_put(
            np.concatenate([np.asarray(in_maps[c][n]) for c in range(NCORES)], axis=0),
            shard)
        for n in in_names
    ]
    def mkzeros():
        return [jax.device_put(np.zeros((NCORES * z.shape[0], *z.shape[1:]), z.dtype), shard)
                for z in zero_outs]
    zsets = [mkzeros() for _ in range(iters)]
    jax.block_until_ready(zsets)
    # warm
    o = sharded(*concat_in, *zsets[0]); jax.block_until_ready(o)
    times = []
    for i in range(1, iters):
        t0 = time.time()
        o = sharded(*concat_in, *zsets[i])
        jax.block_until_ready(o)
        times.append(time.time() - t0)
    times.sort()
    return times[len(times) // 2], times


def kernel_traced(**inputs):
    """Dev helper (not used by the grader): run with NTFF tracing, return
    (exec_time_ns, trace_path)."""
    nc = _get_nc()
    in_maps = _host_prep(**inputs)
    res = run_bass_kernel_spmd(nc, in_maps, core_ids=list(range(NCORES)), trace=True)
    trace_path = res.instructions_and_trace[1] if res.instructions_and_trace else None
    return res.exec_time_ns, trace_path


# revision 43
# speedup vs baseline: 1.0648x; 1.0648x over previous
"""Trainium2 Bass kernel for nn_Decoder_9715216023709.

Computes (lambda_src [512,20000], lambda_dst [512,20000], return_time_pred [512])
for the Hawkes-process decoder, sharded over 8 NeuronCores:
  - the [B,N] lambda branches are sharded along N (2500 nodes per core,
    per-batch params replicated),
  - the [S,B] time-integration branch is sharded along B (64 rows per core),
  - outputs are disjoint, nothing is all-reduced.

Key algebraic restructure: softplus((a[b] + v_e[n]) * ipsi_e) has a separable
exponential, exp(.) = A[b] * V_e[n] with A = exp(a*ipsi) per batch and
V_e = exp(ipsi_e * v_e) per node (both O(B)/O(N), precomputed on the host).
The per-element device work is then a single ACT Ln pass:
  lambda = psi * Ln(1 + A[b] * V_{et(b)}[n])
with the event-type selection of V done by a K=4 bf16 hi/lo matmul
(masks x [Vhi; Vlo]) and A[b] applied as the ACT per-partition scale.
The same trick removes the per-element exp from the [S,B] intensity.

Device layout per core:
  lambda:  partitions = batch (4 tiles of 128), free = node chunks
           (1024|1024|452, PSUM-bank aligned; matmul N<=512 into slices).
           matmul -> ACT Ln(scale*x+1) -> DVE *psi -> DMA out.
  integ:   partitions = (half, b_local), s split into two 2501-halves.
           F-table [128,2501] from host; ACT Ln, DVE cumsum via
           tensor_tensor_scan, cross-half fixup via a tiny DMA, ACT Exp,
           fused multiply-reduce for the trapezoid sum.
"""

import numpy as np

import concourse.bass as bass
import concourse.mybir as mybir
import concourse.tile as tile
from concourse import bacc
from concourse.bass_utils import run_bass_kernel_spmd

FP = mybir.dt.float32
BF = mybir.dt.bfloat16
AF = mybir.ActivationFunctionType
ALU = mybir.AluOpType

NCORES = 8
N, B, D, S = 20000, 512, 32, 5001
NS = N // NCORES          # 2500 nodes per core
BS = B // NCORES          # 64 batch rows per core (integration branch)
H = 2501                  # integration half length (2*H = 5002 >= S)
NT = B // 128             # 4 batch tiles of 128 partitions
TDM = 5000.0
# lambda free-dim chunks: PSUM-bank aligned (bank = 512 f32)
CHUNKS = [(0, 1024), (1024, 1024), (2048, 452)]

_ONE_ACT_SET = "natural_log_exp_and_others"


def _build_nc() -> bass.Bass:
    nc = bacc.Bacc("TRN2", debug=False, target_bir_lowering=False)

    # packed bf16 inputs: [vexp_s | vexp_d | masks4 | ftab | imasks]
    BFW = 2 * NS + B + 2 * H + BS
    binp = nc.dram_tensor("binp", [4, BFW], BF, kind="ExternalInput")
    # packed f32 per-partition params: [lpar(12) | ipar(4)]
    pars = nc.dram_tensor("pars", [128, 16], FP, kind="ExternalInput")
    lam_s = nc.dram_tensor("lam_s", [B, NS], FP, kind="ExternalOutput")
    lam_d = nc.dram_tensor("lam_d", [B, NS], FP, kind="ExternalOutput")
    rtp = nc.dram_tensor("rtp", [BS, 1], FP, kind="ExternalOutput")

    with tile.TileContext(nc) as tc:
        with (
            tc.tile_pool(name="const", bufs=1) as cpool,
            tc.tile_pool(name="work", bufs=8) as wpool,
            tc.tile_pool(name="big", bufs=1) as bigpool,
            tc.tile_pool(name="xps", bufs=3, space="PSUM") as ppool,
            tc.tile_pool(name="xps_s", bufs=2, space="PSUM") as pspool,
        ):
            # ---- input loads: 2 packed DMAs -------------------------------
            binp_sb = cpool.tile([4, BFW], BF, tag="binp")
            nc.sync.dma_start(binp_sb[:], binp[:])
            pars_sb = cpool.tile([128, 16], FP, tag="pars")
            nc.sync.dma_start(pars_sb[:], pars[:])
            vexp_s_sb = binp_sb[:, 0:NS]
            vexp_d_sb = binp_sb[:, NS : 2 * NS]
            masks_sb = binp_sb[:, 2 * NS : 2 * NS + B]
            ftab_sb = binp_sb[:, 2 * NS + B : 2 * NS + B + 2 * H]
            imasks_sb = binp_sb[:, 2 * NS + B + 2 * H : 2 * NS + B + 2 * H + BS]
            lpar_sb = pars_sb[:, 0:12]
            ipar_sb = pars_sb[:, 12:16]

            # zero-dep ACT op: hoists the activation-table load to t=0
            warm = cpool.tile([1, 1], FP, tag="warm")
            nc.scalar.activation(warm[:], warm[:], AF.Ln, bias=1.0, scale=0.0)

            # ---- lambda branches ------------------------------------------
            # lpar columns: [A_src(t) x4 | A_dst(t) x4 | psi(t) x4]
            def lam_group(t):
                for c0, cw in CHUNKS:
                    for lamout, vexp_sb, acol0 in ((lam_s, vexp_s_sb, 0), (lam_d, vexp_d_sb, 4)):
                        pool = ppool if cw > 512 else pspool
                        xps = pool.tile([128, cw], FP, tag=f"xps{cw}")
                        for off in range(0, cw, 512):
                            w = min(512, cw - off)
                            nc.tensor.matmul(
                                xps[:, off : off + w],
                                masks_sb[:, t * 128 : (t + 1) * 128],
                                vexp_sb[:, c0 + off : c0 + off + w],
                                start=True,
                                stop=True,
                            )
                        act_sb = wpool.tile([128, cw], FP, tag=f"act{cw}")
                        nc.scalar.activation(
                            act_sb[:],
                            xps[:],
                            AF.Ln,
                            bias=1.0,
                            scale=lpar_sb[:, acol0 + t : acol0 + t + 1],
                        )
                        out_sb = wpool.tile([128, cw], FP, tag=f"out{cw}")
                        nc.vector.tensor_scalar_mul(
                            out_sb[:], act_sb[:], lpar_sb[:, 8 + t : 9 + t]
                        )
                        nc.sync.dma_start(
                            lamout[t * 128 : (t + 1) * 128, c0 : c0 + cw], out_sb[:]
                        )

            lam_group(0)
            # ---- time-integration branch ----------------------------------
            # ipar columns: [G | psi | H*h | unused]
            sbf = bigpool.tile([128, H], FP, tag="sbf")
            nc.gpsimd.iota(sbf[:], pattern=[[1, H]], base=0, channel_multiplier=0,
                           allow_small_or_imprecise_dtypes=True)

            # trapezoid weights wl[s] = s, except wl[5000]=2500, wl[>=5001]=0
            wl = bigpool.tile([128, H], FP, tag="wl")
            nc.gpsimd.tensor_scalar_add(wl[:], sbf[:], ipar_sb[:, 2:3])
            nc.vector.memset(wl[64:128, H - 2 : H - 1], 2500.0)
            nc.vector.memset(wl[64:128, H - 1 : H], 0.0)

            sp = bigpool.tile([128, H], FP, tag="sp")
            for c0, cw in ((0, 1024), (1024, 1024), (2048, 453)):
                cwp = cw if cw % 512 == 0 else ((cw // 512) + 1) * 512
                xt = ppool.tile([128, cwp], FP, tag="xps1024")
                for off in range(0, cw, 512):
                    w = min(512, cw - off)
                    for h in (0, 1):
                        nc.tensor.matmul(
                            xt[64 * h : 64 * h + 64, off : off + w],
                            imasks_sb,
                            ftab_sb[:, H * h + c0 + off : H * h + c0 + off + w],
                            start=True,
                            stop=True,
                        )
                nc.scalar.activation(sp[:, c0 : c0 + cw], xt[:, :cw], AF.Ln, bias=1.0,
                                     scale=ipar_sb[:, 0:1])
            intens = bigpool.tile([128, H], FP, tag="int")
            nc.gpsimd.tensor_scalar_mul(intens[:], sp[:], ipar_sb[:, 1:2])

            cum = bigpool.tile([128, H], FP, tag="cum")
            nc.vector.tensor_tensor_scan(
                cum[:], intens[:], intens[:], 0.0, op0=ALU.add, op1=ALU.bypass
            )
            # carry half-0 totals into half-1 rows
            fixt = cpool.tile([128, 1], FP, tag="fixt")
            nc.gpsimd.dma_start(fixt[64:128, :], cum[0:64, H - 1 : H])
            nc.gpsimd.tensor_scalar_add(cum[64:128, :], cum[64:128, :], fixt[64:128, :])

            expc = bigpool.tile([128, H], FP, tag="expc")
            nc.scalar.activation(expc[:], cum[:], AF.Exp, bias=0.0, scale=-1.0)
            wint = bigpool.tile([128, H], FP, tag="wint")
            nc.vector.tensor_mul(wint[:], intens[:], wl[:])
            t2 = bigpool.tile([128, H], FP, tag="t2")
            nc.vector.tensor_mul(t2[:], wint[:], expc[:])
            rsum = cpool.tile([128, 1], FP, tag="rsum")
            nc.vector.tensor_reduce(rsum[:], t2[:], axis=mybir.AxisListType.X, op=ALU.add)
            rtop = cpool.tile([128, 1], FP, tag="rtop")
            nc.gpsimd.dma_start(rtop[0:64, :], rsum[64:128, :])
            rout = cpool.tile([64, 1], FP, tag="rout")
            nc.vector.tensor_add(rout[:], rsum[0:64, :], rtop[0:64, :])
            nc.gpsimd.dma_start(rtp[:], rout[:])


            for t in range(1, NT):
                lam_group(t)

    # Bias activation-table selection to the single exp+ln set so the
    # compiler emits one table load instead of alternating per function.
    import concourse.bacc as bacc_mod
    orig_gat = bacc_mod.get_activation_tables
    def _gat_one_set(arch):
        t = orig_gat(arch)
        return {k: (v if k == _ONE_ACT_SET else set()) for k, v in t.items()}
    bacc_mod.get_activation_tables = _gat_one_set
    try:
        nc.compile()
    finally:
        bacc_mod.get_activation_tables = orig_gat
    return nc


_NC_CACHE: list = []


def _get_nc() -> bass.Bass:
    if not _NC_CACHE:
        _NC_CACHE.append(_build_nc())
    return _NC_CACHE[0]


def _host_prep(all_embeddings, assoc, src, pos_dst, neg_dst, last_update,
               cur_time, et, W0, b0, W1, b1, psi, alpha, w_t):
    import ml_dtypes
    bf16 = ml_dtypes.bfloat16
    f8 = np.float64

    emb = np.asarray(all_embeddings, f8)
    assoc = np.asarray(assoc, np.int64)
    src = np.asarray(src, np.int64)
    pos_dst = np.asarray(pos_dst, np.int64)
    last_update = np.asarray(last_update, f8)
    cur_time = np.asarray(cur_time, f8)
    etb = (np.asarray(et) > 0).astype(np.int64)
    W0 = np.asarray(W0, f8)
    W1 = np.asarray(W1, f8)
    b0f, b1f = float(np.asarray(b0).reshape(-1)[0]), float(np.asarray(b1).reshape(-1)[0])
    psi = np.asarray(psi, f8)
    alpha = np.asarray(alpha, f8)
    w_t = np.asarray(w_t, f8)

    W0u, W0v = W0[0, :D], W0[0, D:]
    W1u, W1v = W1[0, :D], W1[0, D:]

    ipsi2 = 1.0 / (psi + 1e-7)           # per event type [2]
    psi_e, alpha_e, wt_e = psi[etb], alpha[etb], w_t[etb]
    inv_psi = ipsi2[etb]
    z_src = emb[assoc[src]]
    z_dst = emb[assoc[pos_dst]]
    td_src = cur_time - last_update[assoc[src]]
    td_dst = cur_time - last_update[assoc[pos_dst]]

    a_src = np.where(etb == 1, z_src @ W1u + b1f, z_src @ W0u + b0f) \
        + alpha_e * np.exp(-wt_e * td_src / TDM)
    a_dst = np.where(etb == 1, z_dst @ W1v + b1f, z_dst @ W0v + b0f) \
        + alpha_e * np.exp(-wt_e * td_dst / TDM)
    gb = np.where(etb == 1, z_src @ W1u + z_dst @ W1v + b1f,
                  z_src @ W0u + z_dst @ W0v + b0f)

    A_src = np.exp(a_src * inv_psi)
    A_dst = np.exp(a_dst * inv_psi)
    G = np.exp(gb * inv_psi)

    lpar = np.zeros((128, 12), np.float32)
    for t in range(NT):
        sl = slice(128 * t, 128 * (t + 1))
        lpar[:, t] = A_src[sl]
        lpar[:, 4 + t] = A_dst[sl]
        lpar[:, 8 + t] = psi_e[sl]

    m0, m1 = (etb == 0).astype(f8), (etb == 1).astype(f8)
    masks_np = np.stack([m0, m1, m0, m1]).astype(bf16)  # [4, B]

    # per-node exp tables V_e = exp(ipsi_e * (emb @ W)), split hi/lo in bf16
    def _vexp(Wa, Wb):  # event-type 0 uses Wa, 1 uses Wb
        V0 = np.exp(ipsi2[0] * (emb @ Wa))
        V1 = np.exp(ipsi2[1] * (emb @ Wb))
        return V0, V1

    V0s, V1s = _vexp(W0v, W1v)   # lambda_src selects node_v
    V0d, V1d = _vexp(W0u, W1u)   # lambda_dst selects node_u

    # integration F table: F[s,e] = exp(c_e * exp(-w_e*s/TDM)), c_e=ipsi_e*alpha_e
    s_all = np.arange(2 * H, dtype=f8)
    Ftab = np.exp((ipsi2 * alpha)[None, :] * np.exp(-w_t[None, :] * s_all[:, None] / TDM))  # [2H, 2]
    fhi = Ftab.astype(bf16)                          # [2H, 2]
    flo = (Ftab - fhi.astype(f8)).astype(bf16)
    ftab_np = np.stack([fhi[:, 0], fhi[:, 1], flo[:, 0], flo[:, 1]]).astype(bf16)  # [4, 2H]

    def _hilo4(V0, V1, sl):
        rows = np.stack([V0[sl], V1[sl]])                  # [2, NS] f64
        hi = rows.astype(bf16)
        lo = (rows - hi.astype(f8)).astype(bf16)
        return np.concatenate([hi, lo], axis=0)            # [4, NS]: V0hi,V1hi,V0lo,V1lo

    in_maps = []
    for i in range(NCORES):
        nsl = slice(NS * i, NS * (i + 1))
        bsl = slice(BS * i, BS * (i + 1))
        ipar = np.zeros((128, 4), np.float32)
        for h in (0, 1):
            rows = slice(64 * h, 64 * (h + 1))
            ipar[rows, 0] = G[bsl]
            ipar[rows, 1] = psi_e[bsl]
            ipar[rows, 2] = H * h
        im0 = (etb[bsl] == 0).astype(f8)
        im1 = (etb[bsl] == 1).astype(f8)
        imasks_np = np.stack([im0, im1, im0, im1]).astype(bf16)  # [4, BS]
        binp = np.concatenate(
            [_hilo4(V0s, V1s, nsl), _hilo4(V0d, V1d, nsl), masks_np,
             ftab_np, imasks_np], axis=1).astype(bf16)           # [4, BFW]
        pars = np.concatenate([lpar, ipar], axis=1).astype(np.float32)  # [128, 16]
        in_maps.append({"binp": binp, "pars": pars})
    return in_maps


def kernel(**inputs) -> tuple:
    nc = _get_nc()
    in_maps = _host_prep(**inputs)
    res = run_bass_kernel_spmd(nc, in_maps, core_ids=list(range(NCORES)))
    lam_s = np.concatenate([r["lam_s"] for r in res.results], axis=1)
    lam_d = np.concatenate([r["lam_d"] for r in res.results], axis=1)
    rtp = np.concatenate([r["rtp"][:, 0] for r in res.results])
    return (lam_s, lam_d, rtp)


def kernel_timed(iters=8, **inputs):
    """Dev helper: dispatch the compiled NEFF `iters` times with
    device-resident inputs and report (median_delta_s, times). The delta
    between consecutive async executions approximates per-execution device
    time + dispatch overhead."""
    import time
    import jax
    from jax.sharding import Mesh, PartitionSpec, NamedSharding
    from jax.experimental.shard_map import shard_map
    from concourse import bass2jax

    nc = _get_nc()
    in_maps = _host_prep(**inputs)
    bass2jax.install_neuronx_cc_hook()

    partition_name = nc.partition_id_tensor.name if nc.partition_id_tensor else None
    in_names, out_names, out_avals, zero_outs = [], [], [], []
    import concourse.mybir as mb
    for alloc in nc.m.functions[0].allocations:
        if not isinstance(alloc, mb.MemoryLocationSet):
            continue
        name = alloc.memorylocations[0].name
        if alloc.kind == "ExternalInput":
            if name != partition_name:
                in_names.append(name)
        elif alloc.kind == "ExternalOutput":
            shape = tuple(alloc.tensor_shape)
            dtype = mb.dt.np(alloc.dtype)
            out_names.append(name)
            out_avals.append(jax.core.ShapedArray(shape, dtype))
            zero_outs.append(np.zeros(shape, dtype))
    n_params = len(in_names)
    n_outs = len(out_avals)
    all_in_names = list(in_names) + list(out_names)
    if partition_name is not None:
        all_in_names.append(partition_name)

    def _body(*args):
        operands = list(args)
        if partition_name is not None:
            operands.append(bass2jax.partition_id_tensor())
        outs = bass2jax._bass_exec_p.bind(
            *operands,
            out_avals=tuple(out_avals),
            in_names=tuple(all_in_names),
            out_names=tuple(out_names),
            lowering_input_output_aliases=(),
            sim_require_finite=True,
            sim_require_nnan=True,
            nc=nc,
        )
        return tuple(outs)

    devices = jax.devices()[:NCORES]
    mesh = Mesh(np.asarray(devices), ("core",))
    spec = PartitionSpec("core")
    sharded = jax.jit(
        shard_map(_body, mesh=mesh,
                  in_specs=(spec,) * (n_params + n_outs),
                  out_specs=(spec,) * n_outs, check_rep=False),
        donate_argnums=tuple(range(n_params, n_params + n_outs)),
        keep_unused=True,
    )
    shard = NamedSharding(mesh, spec)
    concat_in = [
        jax.device_put(
            np.concatenate([np.asarray(in_maps[c][n]) for c in range(NCORES)], axis=0),
            shard)
        for n in in_names
    ]
    def mkzeros():
        return [jax.device_put(np.zeros((NCORES * z.shape[0], *z.shape[1:]), z.dtype), shard)
                for z in zero_outs]
    zsets = [mkzeros() for _ in range(iters)]
    jax.block_until_ready(zsets)
    # warm
    o = sharded(*concat_in, *zsets[0]); jax.block_until_ready(o)
    times = []
    for i in range(1, iters):
        t0 = time.time()
        o = sharded(*concat_in, *zsets[i])
        jax.block_until_ready(o)
        times.append(time.time() - t0)
    times.sort()
    return times[len(times) // 2], times


def kernel_traced(**inputs):
    """Dev helper (not used by the grader): run with NTFF tracing, return
    (exec_time_ns, trace_path)."""
    nc = _get_nc()
    in_maps = _host_prep(**inputs)
    res = run_bass_kernel_spmd(nc, in_maps, core_ids=list(range(NCORES)), trace=True)
    trace_path = res.instructions_and_trace[1] if res.instructions_and_trace else None
    return res.exec_time_ns, trace_path
